# revision 28
# baseline (speedup 1.0000x reference)
"""Bass/Tile MHA kernel for trn2, sharded over 8 cores as (batch, head-group).

Each core handles one batch b and 3 heads. Inputs are host-prepared (bf16):
  qt, kt, vt : [D, S] bf16   — Q[b].T etc. (transposed on host)
  mt         : [S, S] bf16   — mask[b,0].T  (mt[k, q] = mask[b,0,q,k]), 0/1
  wqt, wkt, wvt : [D, 3*DK] bf16 — W_X.T[:, head_cols]
  wot        : [3*DK, D] bf16    — W_O.T[head_rows, :]
Output:
  out : [S, D] bf16 — partial output (sum over the 4 head-groups of a batch
        gives the final output rows for that batch).

Engine budget per core (targets): PE ~120us, ACT ~110us (96 exp over
2-bank PSUM groups), DVE ~90us, GpSimd broadcast only, DMA ~22MB.
"""

import numpy as np

import concourse.bass as bass
import concourse.bacc as bacc
import concourse.tile as tile
import concourse.mybir as mybir

F32 = mybir.dt.float32
F32R = mybir.dt.float32r
BF16 = mybir.dt.bfloat16
F8E4 = mybir.dt.float8e4
AF = mybir.ActivationFunctionType
ALU = mybir.AluOpType

D = 768
DK = 64
NH = 3          # heads per core
HD = NH * DK    # 192


def build_mha_nc(S=2048, n_cores=8, reps=1, bcast_dram=False, debug_taps=False,
                 WARMKEEP=False):
    ST = S // 128   # s-tiles (attention k-tiles)
    QQ = S // 512   # q quarters
    KT6 = D // 128  # contraction tiles for projections
    KTP = ST // 2   # k-tile pairs (one exp group = 2 k-tiles)

    nc = bacc.Bacc("TRN2", target_bir_lowering=False, debug=False,
                   num_devices=n_cores)

    qt_d = nc.dram_tensor("qt", [D, S], BF16, kind="ExternalInput")
    kt_d = nc.dram_tensor("kt", [D, S], BF16, kind="ExternalInput")
    vt_d = nc.dram_tensor("vt", [D, S], BF16, kind="ExternalInput")
    mt_d = nc.dram_tensor("mt", [S, S], BF16, kind="ExternalInput")
    wqt_d = nc.dram_tensor("wqt", [D, HD], BF16, kind="ExternalInput")
    wkt_d = nc.dram_tensor("wkt", [D, HD], BF16, kind="ExternalInput")
    wvt_d = nc.dram_tensor("wvt", [D, HD], BF16, kind="ExternalInput")
    wot_d = nc.dram_tensor("wot", [256, D], BF16, kind="ExternalInput")
    out_d = nc.dram_tensor("out", [S, D], BF16, kind="ExternalOutput")
    if bcast_dram:
        scratch_d = nc.dram_tensor("recip_scratch", [NH, S], F32)
    if debug_taps:
        dbg_qt = nc.dram_tensor("dbg_qt", [128, S], F32R, kind="ExternalOutput")
        dbg_kt = nc.dram_tensor("dbg_kt", [128, S], F32R, kind="ExternalOutput")
        dbg_ps = nc.dram_tensor("dbg_ps", [128, 2, 512], F32, kind="ExternalOutput")
        dbg_ex = nc.dram_tensor("dbg_ex", [128, 2, 512], BF16, kind="ExternalOutput")
        dbg_em = nc.dram_tensor("dbg_em", [128, 2, 512], BF16, kind="ExternalOutput")
        dbg_avs = nc.dram_tensor("dbg_avs", [128, NH, 512], F32, kind="ExternalOutput")
        dbg_bc = nc.dram_tensor("dbg_bc", [64, NH, 512], F32, kind="ExternalOutput")
        dbg_vsb = nc.dram_tensor("dbg_vsb", [128, 16, NH, DK + 1], BF16, kind="ExternalOutput")
        dbg_msb = nc.dram_tensor("dbg_msb", [128, 2, 512], BF16, kind="ExternalOutput")

    with tile.TileContext(nc) as tc:
      for _rep in range(reps):
        with tc.tile_pool(name="perm", bufs=1) as perm:
            # ---- persistent SBUF tensors ----
            qT_a = perm.tile([128, S], BF16, tag="qT_a")   # h0: 0-63, h1: 64-127
            qT_b = perm.tile([128, S], BF16, tag="qT_b")   # h2 (dup at 64-127)
            kT_a = perm.tile([128, S], BF16, tag="kT_a")
            kT_b = perm.tile([128, S], BF16, tag="kT_b")
            v_sb = perm.tile([128, ST, NH, DK + 1], BF16, tag="v_sb")
            m_sb = perm.tile([128, ST, S], BF16, tag="m_sb")
            attnT_a = perm.tile([128, S], BF16, tag="attnT_a")
            attnT_b = perm.tile([128, S], BF16, tag="attnT_b")

            ebias = perm.tile([128, 1], F32, tag="ebias")
            nc.vector.memset(ebias[:], -2.0)
            nc.vector.memset(v_sb[:], 1.0)
            nc.vector.memset(attnT_b[:], 0.0)

            # ---- phase 1: q/k projections (v-proj runs inside qq0) ----
            wv_sb = perm.tile([128, KT6, HD], BF16, tag="wv")
            v_raw = perm.tile([128, KT6, S], BF16, tag="v_raw")
            wot_a = perm.tile([128, D], BF16, tag="wot_a")
            wot_b = perm.tile([128, D], BF16, tag="wot_b")
            with (
                tc.tile_pool(name="weights", bufs=1) as wpool,
                tc.tile_pool(name="raw", bufs=2) as raw_pool,
                tc.tile_pool(name="ps_proj", bufs=3, space="PSUM") as psum_proj,
            ):
                wq_sb = wpool.tile([128, KT6, HD], BF16, tag="wq")
                wk_sb = wpool.tile([128, KT6, HD], BF16, tag="wk")
                nc.sync.dma_start(
                    wq_sb[:], wqt_d.ap().rearrange("(o p) m -> p o m", p=128))
                nc.sync.dma_start(
                    wk_sb[:], wkt_d.ap().rearrange("(o p) m -> p o m", p=128))
                nc.sync.dma_start(
                    wv_sb[:], wvt_d.ap().rearrange("(o p) m -> p o m", p=128))

                def load_raw(x_dram, name):
                    x_raw = raw_pool.tile([128, KT6, S], BF16, tag="raw",
                                          name=name)
                    x_t = x_dram.ap().rearrange("(o p) s -> p o s", p=128)
                    for kt in range(KT6):
                        nc.sync.dma_start(x_raw[:, kt, :], x_t[:, kt, :])
                    return x_raw

                PW_PROJ = min(1024, S)

                def project_T(x_raw, w_sb, dst_a, dst_b):
                    # dst_a[0:128] = (x@w[:,0:128]).T ; dst_b = (x@w[:,128:]).T
                    PW = PW_PROJ
                    for mt_i, (dst, mw) in enumerate([(dst_a, 128),
                                                      (dst_b, 64)]):
                        for w in range(S // PW):
                            ps = psum_proj.tile([128, PW], F32, tag="ps_proj",
                                                name="ps_proj")
                            for kt in range(KT6):
                                for half in range(PW // 512):
                                    nc.tensor.matmul(
                                        ps[:mw, half * 512:(half + 1) * 512],
                                        w_sb[:, kt,
                                             mt_i * 128: mt_i * 128 + mw],
                                        x_raw[:, kt, w * PW + half * 512:
                                              w * PW + (half + 1) * 512],
                                        start=(kt == 0), stop=(kt == KT6 - 1))
                            # scalar engine is otherwise idle in this phase
                            nc.scalar.copy(
                                dst[:mw, w * PW:(w + 1) * PW], ps[:mw, :])
                            if mw == 64:
                                # duplicate h2 at partitions 64-127 so its
                                # score MMs can alternate PE row-tiles
                                nc.scalar.copy(
                                    dst[64:128, w * PW:(w + 1) * PW],
                                    ps[:mw, :])

                q_raw = load_raw(qt_d, "q_raw")
                project_T(q_raw, wq_sb, qT_a, qT_b)
                k_raw = load_raw(kt_d, "k_raw")
                project_T(k_raw, wk_sb, kT_a, kT_b)

                # v raw + mask loads (consumed below / in phase 2; after
                # q/k loads so they do not starve them)
                v_t = vt_d.ap().rearrange("(o p) s -> p o s", p=128)
                for kt in range(KT6):
                    nc.sync.dma_start(v_raw[:, kt, :], v_t[:, kt, :])
                mt_r = mt_d.ap().rearrange("(t p) q -> p t q", p=128)
                for t in range(ST):
                    nc.sync.dma_start(m_sb[:, t, :], mt_r[:, t, :])
                nc.sync.dma_start(wot_a[:], wot_d.ap()[0:128, :])
                nc.sync.dma_start(wot_b[:], wot_d.ap()[128:256, :])

                # v projection: v[s, 3*64] s-major, bf16
                for st in range(ST):
                    psv = psum_proj.tile([128, PW_PROJ], F32, tag="ps_proj",
                                         name=f"psv{st}")
                    for kt in range(KT6):
                        nc.tensor.matmul(
                            psv[:, 0:HD],
                            v_raw[:, kt, st * 128:(st + 1) * 128],
                            wv_sb[:, kt, :],
                            start=(kt == 0), stop=(kt == KT6 - 1))
                    nc.vector.tensor_copy(
                        v_sb[:, st, :, 0:DK],
                        psv[:, 0:HD].rearrange("p (h d) -> p h d", h=NH))

            # ---- phase 2: attention (+ interleaved output projection) ----
            # Score MMs alternate PE row-tiles T0/T8 (h0 at partitions 0-63,
            # h1 at 64-127, h2 duplicated on both) so LDWEIGHTS pulls ahead
            # of in-flight MMs and the PE stays HAM-warm. avs is eagerly
            # copied out of PSUM; norm-mult + out-proj are emitted deferred.
            with (
                tc.tile_pool(name="ps_s", bufs=2, space="PSUM") as psum_s_pool,
                tc.tile_pool(name="ps_av", bufs=1, space="PSUM") as psum_av_pool,
                tc.tile_pool(name="expp", bufs=6) as exp_pool,
                tc.tile_pool(name="emp", bufs=6) as em_pool,
                tc.tile_pool(name="avsb", bufs=6) as avsb_pool,
                tc.tile_pool(name="norm", bufs=3) as norm_pool,
                tc.tile_pool(name="outp", bufs=2) as out_pool,
                tc.tile_pool(name="ps_o", bufs=1, space="PSUM") as psum_o_pool,
            ):
                if debug_taps:
                    nc.sync.dma_start(dbg_qt.ap()[:], qT_a[:])
                    nc.sync.dma_start(dbg_kt.ap()[:], kT_a[:])
                    nc.sync.dma_start(dbg_vsb.ap()[:], v_sb[:])
                    nc.sync.dma_start(dbg_msb.ap()[:], m_sb[:, 0:2, 0:512])

                deferred = []

                def make_outproj(qq_i):
                    def emit():
                        for st in range(qq_i * 4, qq_i * 4 + 4):
                            ob = out_pool.tile([128, D], BF16, tag="ob",
                                               name=f"ob{st}")
                            for half in range(2):
                                o = half * 384
                                po = psum_o_pool.tile(
                                    [128, 384], F32, tag="po",
                                    name=f"po{st}_{half}")
                                nc.tensor.matmul(
                                    po[:],
                                    attnT_a[:, st * 128:(st + 1) * 128],
                                    wot_a[:, o:o + 384],
                                    start=True, stop=False)
                                nc.tensor.matmul(
                                    po[:],
                                    attnT_b[:, st * 128:(st + 1) * 128],
                                    wot_b[:, o:o + 384],
                                    start=False, stop=True)
                                nc.vector.tensor_copy(ob[:, o:o + 384], po[:])
                            nc.sync.dma_start(
                                out_d.ap()[st * 128:(st + 1) * 128, :], ob[:])
                    return emit

                def make_norm_tt(av_t, bc_t, qq_i, h_i):
                    def emit():
                        q0i = qq_i * 512
                        if h_i < 2:
                            dst = attnT_a[h_i * 64:(h_i + 1) * 64,
                                          q0i:q0i + 512]
                        else:
                            dst = attnT_b[0:64, q0i:q0i + 512]
                        nc.vector.tensor_tensor(dst, av_t[0:DK, :], bc_t[:],
                                                ALU.mult)
                    return emit

                for qq in range(QQ):
                    q0 = qq * 512
                    avs = {}
                    for h in range(NH):
                        avs[h] = psum_av_pool.tile([DK + 1, 512], F32,
                                                   tag=f"avs{h}",
                                                   name=f"avs{qq}_{h}")
                    for ktp in range(KTP):
                        if deferred and ktp in (1, 2, 3, 4):
                            deferred.pop(0)()
                        ktA, ktB = 2 * ktp, 2 * ktp + 1
                        ps = {h: psum_s_pool.tile([128, 2, 512], F32, tag="s",
                                                  name=f"ps{h}")
                              for h in range(NH)}
                        # T0/T8-alternating score MMs
                        seq = [(0, 0, qT_a, kT_a, 0),
                               (1, 0, qT_a, kT_a, 64),
                               (0, 1, qT_a, kT_a, 0),
                               (1, 1, qT_a, kT_a, 64),
                               (2, 0, qT_b, kT_b, 0),
                               (2, 1, qT_b, kT_b, 64)]
                        for h, i, qsrc, ksrc, p0 in seq:
                            kt = ktA if i == 0 else ktB
                            nc.tensor.matmul(
                                ps[h][:, i, :],
                                ksrc[p0:p0 + DK, kt * 128:(kt + 1) * 128],
                                qsrc[p0:p0 + DK, q0:q0 + 512],
                                start=True, stop=True)
                        for h in range(NH):
                            ex = exp_pool.tile([128, 2, 512], BF16, tag="ex",
                                               name=f"ex{h}")
                            nc.scalar.activation(ex[:], ps[h][:], AF.Exp,
                                                 scale=0.125)
                            em = em_pool.tile([128, 2, 512], BF16, tag="em",
                                              name=f"em{h}")
                            nc.vector.tensor_tensor(
                                em[:], ex[:],
                                m_sb[:, ktA:ktA + 2, q0:q0 + 512],
                                ALU.mult)
                            if debug_taps and qq == 0 and ktp == 0 and h == 0:
                                pscp = em_pool.tile([128, 2, 512], F32,
                                                    tag="pscp", name="pscp")
                                nc.vector.tensor_copy(pscp[:], ps[h][:])
                                nc.sync.dma_start(dbg_ps.ap()[:], pscp[:])
                                nc.sync.dma_start(dbg_ex.ap()[:], ex[:])
                                nc.sync.dma_start(dbg_em.ap()[:], em[:])
                            for i in range(2):
                                kt = ktA if i == 0 else ktB
                                nc.tensor.matmul(
                                    avs[h][0:DK + 1, :],
                                    v_sb[:, kt, h, :],
                                    em[:, i, :],
                                    start=(ktp == 0 and i == 0),
                                    stop=(ktp == KTP - 1 and i == 1))

                    for h in range(NH):
                        # eager PSUM->SBUF drain frees the avs bank without
                        # waiting on the reciprocal/broadcast chain
                        av_sb = avsb_pool.tile([DK + 1, 512], F32, tag="avsb",
                                               name=f"avsb{qq}_{h}")
                        nc.vector.tensor_copy(av_sb[:], avs[h][:])
                        rin = norm_pool.tile([1, 512], F32, tag="rin",
                                             name=f"rin{qq}_{h}")
                        nc.vector.tensor_copy(rin[:], avs[h][DK:DK + 1, :])
                        rc = norm_pool.tile([1, 512], F32, tag="rc",
                                            name=f"rc{qq}_{h}")
                        nc.vector.reciprocal_approx_fast(rc[:], rin[:])
                        bc = norm_pool.tile([64, 512], F32, tag="bc",
                                            name=f"bc{qq}_{h}")
                        if bcast_dram:
                            nc.sync.dma_start(
                                scratch_d.ap()[h, q0:q0 + 512], rc[:])
                            nc.sync.dma_start(
                                bc[:],
                                scratch_d.ap()[h, q0:q0 + 512]
                                .partition_broadcast(64))
                        else:
                            nc.gpsimd.partition_broadcast(bc[:], rc[:])
                        if debug_taps and qq == 0 and h == 0:
                            nc.sync.dma_start(dbg_bc.ap()[:, 0, :], bc[:])
                            nc.sync.dma_start(dbg_avs.ap()[:, 0, :], av_sb[:])
                        if qq == QQ - 1:
                            # last qq: no later rounds to hide the work in --
                            # emit now so the tail stays short
                            make_norm_tt(av_sb, bc, qq, h)()
                            if h == NH - 1:
                                make_outproj(qq)()
                        else:
                            deferred.append(make_norm_tt(av_sb, bc, qq, h))
                            if h == NH - 1:
                                deferred.append(make_outproj(qq))

                for fn in deferred:
                    fn()

    nc.compile()
    return nc


def make_in_maps(Q, K, V, mask, W_Q, W_K, W_V, W_O, n_cores=8):
    import ml_dtypes
    bf = ml_dtypes.bfloat16
    in_maps = []
    for c in range(n_cores):
        b, g = divmod(c, 4)
        hs = slice(g * HD, (g + 1) * HD)
        in_maps.append({
            "qt": np.ascontiguousarray(Q[b].T).astype(bf),
            "kt": np.ascontiguousarray(K[b].T).astype(bf),
            "vt": np.ascontiguousarray(V[b].T).astype(bf),
            "mt": np.ascontiguousarray(mask[b, 0].T).astype(bf),
            "wqt": np.ascontiguousarray(W_Q.T[:, hs]).astype(bf),
            "wkt": np.ascontiguousarray(W_K.T[:, hs]).astype(bf),
            "wvt": np.ascontiguousarray(W_V.T[:, hs]).astype(bf),
            "wot": np.concatenate([
                np.ascontiguousarray(W_O.T[hs, :]),
                np.zeros((256 - HD, D), np.float32)]).astype(bf),
        })
    return in_maps


def combine_outputs(partials):
    ps = [np.asarray(p, dtype=np.float32) for p in partials]
    b0 = ps[0] + ps[1] + ps[2] + ps[3]
    b1 = ps[4] + ps[5] + ps[6] + ps[7]
    return np.stack([b0, b1])


_NC_CACHE = {}


def _get_nc(reps=1):
    key = ("nc", reps)
    if key not in _NC_CACHE:
        _NC_CACHE[key] = build_mha_nc(S=2048, n_cores=8, reps=reps)
    return _NC_CACHE[key]


def kernel(Q, K, V, mask, W_Q, W_K, W_V, W_O, _reps=1):
    from concourse.bass_utils import run_bass_kernel_spmd
    nc = _get_nc(_reps)
    in_maps = make_in_maps(np.asarray(Q, np.float32), np.asarray(K, np.float32),
                           np.asarray(V, np.float32), np.asarray(mask),
                           np.asarray(W_Q, np.float32), np.asarray(W_K, np.float32),
                           np.asarray(W_V, np.float32), np.asarray(W_O, np.float32))
    res = run_bass_kernel_spmd(nc, in_maps, core_ids=list(range(8)))
    out = combine_outputs([res.results[c]["out"] for c in range(8)])
    return out.astype(np.float32)
